# revision 25
# baseline (speedup 1.0000x reference)
"""Trainium2 Bass kernel for the bilevel logit-assignment flow problem.

Reference math (N=384, cutoff-2 paths):
    A = (adj > 0) & ~eye
    E = A * exp(-lam * dist)                        # "edge weight" matrix
    Z = E + offdiag(E @ E)                          # softmax denominator
    W = demand / Z    (demand = od offdiag; od > 0 and Z > 0 off-diag
                       for this input family; diag handled by od=0 and
                       Z-diag = round-trip mass > 0)
    flows = W*E + E*(W @ E^T) + E*(E^T @ W)

Sharding with node-relabeling: the computation is equivariant under a
symmetric permutation of nodes, so core i receives all matrices rolled
by -48*i on both axes. Its origin slice is then ALWAYS rows 0..47, and
its flow contribution lands in the tile-0 partitions 0..47 of the p3
output. Host un-rolls the outputs and sums in f32.

Device-side structure (E, E^T are computed on the HOST, shipped bf16):
    estid   = [EsT tiles | identity]: the lhsT for Es @ E is a free
              host-provided column-slice of E^T; the 48x48 identity
              rides along in slot 3.  estid gates every EEs lhsT, so
              it is the FIRST DMA on its queue.
    EEs     = identb@Es + Es @ E on the PE (f32 psum), in column halves
              so the reciprocal/W chain starts while the second half
              accumulates.  diag(EEs) = round-trip mass > 0, od's diag
              host-zeroed: no eps needed.
    W       = od (.) reciprocal_approx_fast(EEs), in column halves.
    etin'   = I + E^T, so T2 = W @ etin' = W + W @ E^T needs no
              identity matmul on the PE.
    p3      = E (.) (Es^T @ W  +  rows0:48[ T2 ])
              tiles 1, 2 and tile-0 partitions 64:128 ship early; the
              T2 slice (partitions 0:64, 49KB) ships last.
Inputs stream on the two hardware DGE queues (SP + Activation), in
need-by order; the GpSimd software queue is an order of magnitude
slower and is unused.  All matmul operands bf16 (f32 psum); outputs
f16; host sums in f32.
"""

import ml_dtypes
import numpy as np

import concourse.bass as bass
import concourse.mybir as mybir
import concourse.tile as tile
from concourse import bacc
from concourse.bass_utils import run_bass_kernel_spmd

N = 384
NCORES = 8
S = N // NCORES  # 48 origins per core
P = 128
NT = N // P  # 3 partition tiles
H = N // 2
HP = P // 2

F32 = mybir.dt.float32
F16 = mybir.dt.float16
BF16 = mybir.dt.bfloat16

BF = ml_dtypes.bfloat16

HALVES = ((0, H), (H, N))


def build_program() -> bass.Bass:
    nc = bacc.Bacc(
        "TRN2",
        target_bir_lowering=False,
        debug=False,
        num_devices=NCORES,
        enable_asserts=False,
    )

    ein_d = nc.dram_tensor("ein", [P, NT, N], BF16, kind="ExternalInput")
    etin_d = nc.dram_tensor("etin", [P, NT, N], BF16, kind="ExternalInput")
    estid_d = nc.dram_tensor("estid", [P, NT + 1, S], BF16, kind="ExternalInput")
    odt_d = nc.dram_tensor("odt", [S, N], BF16, kind="ExternalInput")
    p3_d = nc.dram_tensor("p3_t", [P, NT, N], F16, kind="ExternalOutput")

    with tile.TileContext(nc) as tc:
        with (
            tc.tile_pool(name="sb", bufs=1) as sb,
            tc.tile_pool(name="pst", bufs=3, space="PSUM") as pst,
            tc.tile_pool(name="psacc", bufs=1, space="PSUM") as psacc,
            tc.tile_pool(name="psp3", bufs=1, space="PSUM") as psp3,
        ):
            ein = sb.tile([P, NT, N], BF16)
            etin = sb.tile([P, NT, N], BF16)
            estid = sb.tile([P, NT + 1, S], BF16)
            ods = sb.tile([S, N], BF16)

            # ---- input DMA, need-by order across the two HW queues ----
            nc.scalar.dma_start(estid[:], estid_d[:])
            nc.sync.dma_start(ein[:, 0, :], ein_d[:, 0, :])
            nc.scalar.dma_start(ein[:, 1, :], ein_d[:, 1, :])
            nc.sync.dma_start(ein[:, 2, :], ein_d[:, 2, :])
            nc.scalar.dma_start(ods[:], odt_d[:])
            nc.sync.dma_start(etin[:, 0, :], etin_d[:, 0, :])
            nc.scalar.dma_start(etin[:, 1, :], etin_d[:, 1, :])
            nc.sync.dma_start(etin[:, 2, :], etin_d[:, 2, :])

            identb = estid[0:S, NT, :]
            Es = ein[0:S, 0, :]  # origin rows 0..47 in rolled space

            # ---- EEs = Es + Es @ E in column halves.  The halves live in
            #      SEPARATE psum tiles so the first reciprocal only waits
            #      on the first half's accumulation; the ein[2] matmuls
            #      (the last tile to arrive) are issued last ----
            EEsh = [
                psacc.tile([S, H], F32, tag=f"EEs{h}", name=f"EEs{h}")
                for h in range(2)
            ]
            for h, (a, b) in enumerate(HALVES):
                EEs = EEsh[h]
                nc.tensor.matmul(
                    EEs[:], estid[:, 0, :], ein[:, 0, a:b],
                    start=True, stop=False,
                )
                nc.tensor.matmul(
                    EEs[:], estid[:, 1, :], ein[:, 1, a:b],
                    start=False, stop=False,
                )
                nc.tensor.matmul(
                    EEs[:], identb, Es[:, a:b], start=False, stop=False
                )
            for h, (a, b) in enumerate(HALVES):
                nc.tensor.matmul(
                    EEsh[h][:], estid[:, 2, :], ein[:, 2, a:b],
                    start=False, stop=True,
                )

            # ---- W = od (.) recip(EEs), in column halves: reciprocals on
            #      DVE (psum reads), od-multiplies on GpSimd (all-SBUF) so
            #      the two halves pipeline across engines ----
            zinv = sb.tile([S, N], F32)
            W = sb.tile([S, N], BF16)
            for h, (a, b) in enumerate(HALVES):
                nc.vector.reciprocal_approx_fast(zinv[:, a:b], EEsh[h][:])
                nc.gpsimd.tensor_mul(W[:, a:b], ods[:, a:b], zinv[:, a:b])

            out_big = sb.tile([P, NT, N], F16)
            WsT = sb.tile([P, NT, S], BF16)

            # ---- P3 tile 1 (split: starts on the first W half).  The four
            #      P3 psum tiles rotate through 2 banks (P0hi reuses P1's
            #      bank after out-mul 1; P0lo reuses P2's) ----
            P1 = psp3.tile([P, N], F32, tag="PP", bufs=2, name="P1")
            nc.tensor.matmul(
                P1[:, 0:H], Es[:, P : 2 * P], W[:, 0:H], start=True, stop=True
            )
            tp0 = pst.tile([P, S], BF16, tag="tp", bufs=3)
            nc.tensor.transpose(tp0[:], W[:, 0:P], identb)
            nc.scalar.copy(WsT[:, 0, :], tp0[:])
            nc.tensor.matmul(
                P1[:, H:N], Es[:, P : 2 * P], W[:, H:N], start=True, stop=True
            )
            nc.vector.tensor_mul(out_big[:, 1, :], ein[:, 1, :], P1[:])
            nc.sync.dma_start(p3_d[:, 1, :], out_big[:, 1, :])

            # ---- remaining W^T chunks ----
            for c in range(1, NT):
                tp = pst.tile([P, S], BF16, tag="tp", bufs=3)
                nc.tensor.transpose(tp[:], W[:, P * c : P * (c + 1)], identb)
                nc.scalar.copy(WsT[:, c, :], tp[:])

            # ---- P3 tile 2 ----
            P2 = psp3.tile([P, N], F32, tag="PP", bufs=2, name="P2")
            nc.tensor.matmul(P2[:], Es[:, 2 * P : N], W[:], start=True, stop=True)
            nc.vector.tensor_mul(out_big[:, 2, :], ein[:, 2, :], P2[:])
            nc.scalar.dma_start(p3_d[:, 2, :], out_big[:, 2, :])

            # ---- P3 tile 0, partitions 64:128 (no T2 terms) ----
            P0hi = psp3.tile([HP, N], F32, tag="PP", bufs=2, name="P0hi")
            nc.tensor.matmul(P0hi[:], Es[:, HP:P], W[:], start=True, stop=True)
            nc.vector.tensor_mul(out_big[HP:P, 0, :], ein[HP:P, 0, :], P0hi[:])
            nc.scalar.dma_start(p3_d[HP:P, 0, :], out_big[HP:P, 0, :])

            # ---- P3 tile 0, partitions 0:64; T2 = W @ (I + E^T) lands in
            #      the first 48 partitions; shipped last (49KB) ----
            P0lo = psp3.tile([HP, N], F32, tag="PP", bufs=2, name="P0lo")
            nc.tensor.matmul(P0lo[:], Es[:, 0:HP], W[:], start=True, stop=False)
            for c in range(NT):
                nc.tensor.matmul(
                    P0lo[0:S, :], WsT[:, c, :], etin[:, c, :],
                    start=False, stop=(c == NT - 1),
                )
            nc.vector.tensor_mul(out_big[0:HP, 0, :], ein[0:HP, 0, :], P0lo[:])
            nc.sync.dma_start(p3_d[0:HP, 0, :], out_big[0:HP, 0, :])

    nc.compile()
    return nc


_PROGRAM_CACHE: dict = {}


def _get_program(lam: float = 0.0) -> bass.Bass:
    # lam only affects host-side marshaling; one program serves all lam
    if "nc" not in _PROGRAM_CACHE:
        _PROGRAM_CACHE["nc"] = build_program()
    return _PROGRAM_CACHE["nc"]


def _tile_rows(x: np.ndarray) -> np.ndarray:
    """[384, N] row-major -> [128, 3, N] partition-tiled layout."""
    return np.ascontiguousarray(x.reshape(NT, P, -1).transpose(1, 0, 2))


def _untile_rows(x: np.ndarray) -> np.ndarray:
    """[128, 3, N] partition-tiled -> [384, N]."""
    return x.transpose(1, 0, 2).reshape(N, -1)


def make_in_maps(od, adj, dist, lam=1.0):
    eye = np.eye(N, dtype=bool)
    A = adj.astype(bool) & ~eye
    E = np.where(A, np.exp(-lam * dist.astype(np.float64)), 0.0).astype(np.float32)
    odz = od.astype(np.float32).copy()
    np.fill_diagonal(odz, 0.0)
    ident = np.zeros((P, 1, S), np.float32)
    ident[0:S, 0, :] = np.eye(S, dtype=np.float32)
    eyeN = np.eye(N, dtype=np.float32)
    in_maps = []
    for i in range(NCORES):
        r = S * i
        Er = np.roll(E, (-r, -r), axis=(0, 1))
        ein = _tile_rows(Er).astype(BF)
        # etin' = I + E^T: T2 = W @ etin' = W + W @ E^T on one psum pass
        etin = _tile_rows(np.ascontiguousarray(Er.T + eyeN)).astype(BF)
        estid = np.ascontiguousarray(
            np.concatenate(
                [_tile_rows(np.ascontiguousarray(Er.T))[:, :, 0:S], ident],
                axis=1,
            ).astype(BF)
        )
        ods = np.ascontiguousarray(
            np.roll(odz, (-r, -r), axis=(0, 1))[:S]
        ).astype(BF)
        in_maps.append({"ein": ein, "etin": etin, "estid": estid, "odt": ods})
    return in_maps


def gather(results) -> np.ndarray:
    out = np.zeros((N, N), np.float32)
    for i in range(NCORES):
        r = S * i
        p3f = _untile_rows(results[i]["p3_t"]).astype(np.float32)
        out += np.roll(p3f, (r, r), axis=(0, 1))
    return out


def kernel(od, adj, dist, lambda_param, capacity=None, **_unused) -> np.ndarray:
    od = np.ascontiguousarray(np.asarray(od, dtype=np.float32))
    adj = np.ascontiguousarray(np.asarray(adj, dtype=np.int32))
    dist = np.ascontiguousarray(np.asarray(dist, dtype=np.float32))
    lam = float(np.asarray(lambda_param))
    nc = _get_program()
    res = run_bass_kernel_spmd(
        nc, make_in_maps(od, adj, dist, lam), list(range(NCORES))
    )
    return gather(res.results)
